# revision 19
# baseline (speedup 1.0000x reference)
"""Trainium2 Bass kernel for nn_BTRLoss: grayscale morphological opening loss.

Per image: tip = MLP(grid, t) [16x16]; eroded = erosion(image, tip);
recon = dilation(eroded, tip); loss = mean((recon-image)^2) + regularizers.

Strategy: data-parallel over batch (8 images -> 8 cores). On each core the
1024x1024 image is laid out as a 16x8 grid of 64x128 tiles, one tile per SBUF
partition, stored WITH its 79x144 halo so both morphology shift directions are
free-dim offsets. Each of the 256 taps of a morph op is ONE DVE
scalar_tensor_tensor instruction: carry = (window -/+ tip[u,v]) min/max carry.
Two byte-shifted halo copies keep every tap 4B-aligned so fp16 gets the DVE
2x_1P packed mode. The host pre-gathers the haloed input layout (so each input
is a single contiguous DMA); the eroded image round-trips through DRAM on the
single-queue software DGE to rebuild halos (cross-partition exchange), then
the dilation runs the same way. The final squared-diff reduces on-device to
[128,1] partials; the host finishes the mean and adds the tiny tip
regularizer terms.
"""

import numpy as np

try:
    import concourse.bass as bass
except ImportError:
    import sys
    for p in ("/opt/trn_rl_repo", "/root/.axon_site/_ro/trn_rl_repo"):
        if p not in sys.path:
            sys.path.insert(0, p)
    import concourse.bass as bass

import concourse.bacc as bacc
import concourse.tile as tile
from concourse import mybir
from concourse.bass_utils import run_bass_kernel_spmd

# ---- problem geometry (hardcoded per spec) ----
B, H, W = 8, 1024, 1024
K = 16
PAD_BEG = 7          # (K-1)//2
TRG, TCG = 16, 8     # partition tile grid: 16 rows x 8 cols = 128 partitions
TH, TW = 64, 128     # per-partition output tile
HR = TH + K - 1      # 79 halo rows
HC = 144             # halo cols (needs 143; padded to even for alignment)
RB = H + K - 1       # 1039 buffer rows
CB = 1042            # buffer cols: image at col 8, reads reach col 1040
IMG_R0, IMG_C0 = PAD_BEG, PAD_BEG + 1  # image origin inside the DRAM buffer

F32 = mybir.dt.float32
F16 = mybir.dt.float16

# tip grid (matches reference)
_x = np.linspace(-K / 2, K / 2, K, dtype=np.float32)
_X, _Y = np.meshgrid(_x, _x, indexing="ij")
XF = _X.reshape(-1)
YF = _Y.reshape(-1)


def _tip_mlp(t, w1, b1, w2, b2, w3, b3):
    inp = np.stack([XF, YF, np.full(K * K, t, np.float32)], axis=-1)
    h = np.tanh((inp @ w1 + b1).astype(np.float32)).astype(np.float32)
    h = np.tanh((h @ w2 + b2).astype(np.float32)).astype(np.float32)
    return ((h @ w3 + b3)[..., 0]).astype(np.float32)  # [256]


def _assign_engines(n_d, n_g):
    """Per-tap bias-engine assignment for one morph op (256 taps).

    Every tap's min/max runs as a DVE tensor_tensor (2x mode); the bias
    (window +- tip[u,v]) runs on one of three engines: 'D' DVE tensor_scalar
    (4x, needs the 4B-aligned even-v windows), 'A' ACT activation-with-bias
    (alignment-free), 'G' GPSIMD tensor_tensor with a broadcast scalar
    operand. Counts: n_d DVE taps (even-v only), n_g GPSIMD taps, rest ACT.
    Tap 0 initializes the carry via DVE tensor_scalar directly.
    """
    eng = {0: 'D'}
    evens = [k for k in range(2, K * K, 2)]
    rest = []
    for i, k in enumerate(evens):
        if (i * n_d) // len(evens) != ((i + 1) * n_d) // len(evens):
            eng[k] = 'D'
        else:
            rest.append(k)
    rest = sorted(rest + list(range(1, K * K, 2)))
    for i, k in enumerate(rest):
        eng[k] = 'G' if (i * n_g) // len(rest) != ((i + 1) * n_g) // len(rest) \
            else 'A'
    return [eng[k] for k in range(K * K)]


def build_nc(dt=F16, n_d=74, n_g=0, cand_bufs=4):
    nc = bacc.Bacc("TRN2", target_bir_lowering=False)
    ahalo = nc.dram_tensor("ahalo", [128, HR * HC], dt, kind="ExternalInput")
    tips = nc.dram_tensor("tips", [1, K * K], F32, kind="ExternalInput")
    ntips = nc.dram_tensor("ntips", [1, K * K], F32, kind="ExternalInput")
    out_ps = nc.dram_tensor("psum", [128, 1], F32, kind="ExternalOutput")

    sub, add = mybir.AluOpType.subtract, mybir.AluOpType.add
    amin, amax, amult = mybir.AluOpType.min, mybir.AluOpType.max, mybir.AluOpType.mult
    COPY = mybir.ActivationFunctionType.Identity
    assign = _assign_engines(n_d, n_g)

    def morph(halo, tips_act, carry, op0, op1, cpool):
        """carry = reduce_{u,v} (window(u,v) op0 tip[u,v]), reduce = op1."""
        ts_init = {sub: nc.vector.tensor_scalar_sub,
                   add: nc.vector.tensor_scalar_add}[op0]
        for kk in range(K * K):
            u, v = kk // K, kk % K
            win = halo[:, u:u + TH, v:v + TW]
            e = assign[kk]
            if kk == 0:
                ts_init(carry, win, tips_sb[:, 0:1])
                continue
            cand = cpool.tile([128, TH, TW], dt, name="cand")
            if e == 'G':
                sc = tips_sb[:, kk:kk + 1]
                bc = bass.AP(sc.tensor, sc.offset, [sc.ap[0], [0, TH], [0, TW]])
                nc.gpsimd.tensor_tensor(out=cand, in0=win, in1=bc, op=op0)
            elif e == 'A':
                nc.scalar.activation(cand, win, COPY,
                                     bias=tips_act[:, kk:kk + 1], scale=1.0)
            else:
                ts_init(cand, win, tips_sb[:, kk:kk + 1])
            nc.vector.tensor_tensor(out=carry, in0=cand, in1=carry, op=op1)

    with tile.TileContext(nc) as tc:
        with tc.tile_pool(name="sb", bufs=1) as sb, \
             tc.tile_pool(name="cand", bufs=cand_bufs) as cpool, \
             tc.tile_pool(name="dram", bufs=1, space="DRAM") as dram:
            tips_sb = sb.tile([128, K * K], F32)
            nc.sync.dma_start(out=tips_sb,
                              in_=bass.AP(tips, 0, [[0, 128], [1, K * K]]))
            negtips_sb = sb.tile([128, K * K], F32)
            nc.sync.dma_start(out=negtips_sb,
                              in_=bass.AP(ntips, 0, [[0, 128], [1, K * K]]))

            hA = sb.tile([128, HR, HC], dt)
            half = 40 * HC
            nc.sync.dma_start(out=hA[:, 0:40, :], in_=ahalo[:, 0:half])
            nc.scalar.dma_start(out=hA[:, 40:HR, :], in_=ahalo[:, half:HR * HC])
            imgT = sb.tile([128, TH, TW], dt)
            nc.sync.dma_start(
                out=imgT,
                in_=bass.AP(ahalo, PAD_BEG * HC + PAD_BEG,
                            [[HR * HC, 128], [HC, TH], [1, TW]]))

            # ---- erosion: ec = min_{u,v} (window - tip[u,v]) ----
            ec = sb.tile([128, TH, TW], dt)
            morph(hA, negtips_sb, ec, sub, amin, cpool)

            # ---- halo exchange via DRAM round-trip (single SWDGE queue) ----
            epad = dram.tile([RB, CB], dt)
            zrow = sb.tile([128, CB], dt)
            nc.gpsimd.memset(zrow, 0.0)
            for i in range(8):
                nc.gpsimd.dma_start(out=epad[i * 128:(i + 1) * 128, :], in_=zrow[:, :])
            nc.gpsimd.dma_start(out=epad[1024:RB, :], in_=zrow[0:RB - 1024, :])
            # interior: eroded tile (tr,tc) -> rows 7+64*tr, cols 8+128*tc
            for tr in range(TRG):
                nc.sync.dma_start(
                    out=bass.AP(epad.tensor,
                                epad.offset + (IMG_R0 + tr * TH) * CB + IMG_C0,
                                [[TW, TCG], [CB, TH], [1, TW]]),
                    in_=ec[tr * TCG:(tr + 1) * TCG, :, :])
            # reload with halos: partition (tr,tc) rows 64*tr.., cols 128*tc+1..
            eA = sb.tile([128, HR, HC], dt)
            for tr in range(TRG):
                nc.scalar.dma_start(
                    out=eA[tr * TCG:(tr + 1) * TCG, :, :],
                    in_=bass.AP(epad.tensor, epad.offset + 1 + tr * TH * CB,
                                [[TW, TCG], [CB, HR], [1, HC]]))

            # ---- dilation: rc = max_{u,v} (window + tip[u,v]) ----
            rc = sb.tile([128, TH, TW], dt)
            morph(eA, tips_sb, rc, add, amax, cpool)

            # ---- loss: psum[p] = sum over tile of (rc - image)^2 ----
            d = sb.tile([128, TH, TW], dt)
            nc.vector.tensor_tensor(out=d, in0=rc, in1=imgT, op=sub)
            ps = sb.tile([128, 1], F32)
            d2 = sb.tile([128, TH, TW], dt)
            nc.scalar.activation(d2, d, mybir.ActivationFunctionType.Square,
                                 accum_out=ps)
            nc.sync.dma_start(out=bass.AP(out_ps, 0, [[1, 128], [1, 1]]), in_=ps)
    nc.compile()
    return nc


_NC_CACHE = {}


def _get_nc():
    if "nc" not in _NC_CACHE:
        _NC_CACHE["nc"] = build_nc()
    return _NC_CACHE["nc"]


def make_halos(img):
    """Host-side gather of the haloed per-partition layout of one image."""
    buf = np.zeros((RB, CB), np.float16)
    buf[IMG_R0:IMG_R0 + H, IMG_C0:IMG_C0 + W] = img
    win = np.lib.stride_tricks.sliding_window_view(buf, (HR, HC))
    a = win[::TH, 1::TW][:TRG, :TCG].reshape(128, HR * HC)
    return np.ascontiguousarray(a)


def _prep_inputs(images, w1, b1, w2, b2, w3, b3, n):
    bhs, in_maps = [], []
    for b in range(B):
        t = float(n * B + b)
        bh = _tip_mlp(t, w1, b1, w2, b2, w3, b3)
        bhs.append(bh)
        in_maps.append({"ahalo": make_halos(images[b]),
                        "tips": bh[None, :].astype(np.float32),
                        "ntips": (-bh)[None, :].astype(np.float32)})
    return bhs, in_maps


def _finish_loss(bhs, results):
    losses = []
    for b in range(B):
        s = float(np.asarray(results[b]["psum"], np.float64).sum())
        recon = s / (H * W)
        bh = bhs[b]
        tip = bh.reshape(K, K)
        boundary = float(np.mean((bh + 100.0) ** 2))
        reg = float(np.sum(bh ** 2))
        cent = float(np.dot(np.abs(bh), XF)) ** 2 + float(np.dot(np.abs(bh), YF)) ** 2
        avg = float(np.mean(bh)) ** 2
        height = float(np.mean(np.maximum(tip, 0.0) ** 2)) + float(np.max(tip)) ** 2
        losses.append(recon + 0.1 * boundary + 1.0 * height
                      + 1e-4 * reg + 0.1 * avg + 1e-3 * cent)
    return np.array(np.mean(np.asarray(losses, np.float64)), dtype=np.float32)


def _run(inputs, trace=False, **kw):
    images = np.asarray(inputs["images"], np.float32)
    args = [np.asarray(inputs[k], np.float32)
            for k in ("w1", "b1", "w2", "b2", "w3", "b3")]
    n = int(np.asarray(inputs["n"]))
    bhs, in_maps = _prep_inputs(images, *args, n)
    res = run_bass_kernel_spmd(_get_nc(), in_maps, core_ids=list(range(B)),
                               trace=trace, **kw)
    return _finish_loss(bhs, res.results), res


def kernel(**inputs) -> np.ndarray:
    loss, _ = _run(inputs)
    return loss
